# revision 3
# baseline (speedup 1.0000x reference)
"""ARIMA(64, 1, 32) forecast kernel for Trainium2 (Bass/Tile).

Math: with D=1 differencing, the reference's full-series diff is dead code
except its last 64 values (the AR window), and the inverse-differencing
cumsum runs only over the 2048 predictions. The whole output depends on
x[0, -65:, 0] plus the weights:

    d[j]   = xt[j+1] - xt[j]              (last 64 diffs = AR window)
    y_t    = sum_j a_j * y_{t-j} + c      (AR(64), c = b_ar + b_ma, 2048 steps)
    out    = x_last + cumsum(y_1..y_2048)

The sequential AR recurrence is parallelized on the tensor engine via the
65x65 augmented companion matrix C (state = [y_t, y_{t-1}, .., y_{t-63}, 1],
newest first): states s_t = C^t s_0.  Doubling: with S_m = [s_1..s_m] and
C^m, S_2m = [S_m | C^m S_m]; squarings C^{2m} = C^m C^m run alongside
((A A)^T = A^T A^T gives the transposed power without PE transposes).
Predictions are row 0 of S; the final cumsum + x_last is a single
tensor_tensor_scan on the vector engine.  All arithmetic is fp32 on device.

All 8 cores run the identical tiny kernel (the recurrence is replicated per
the sharding hint); core 0's output is returned.
"""

import numpy as np

import concourse.bacc as bacc
import concourse.mybir as mybir
import concourse.tile as tile
from concourse.bass_utils import run_bass_kernel_spmd

F32 = mybir.dt.float32
P = 64          # AR order
STEPS = 2048    # forecast horizon
N_CORES = 8

_CACHE = {}

# dev knobs (ignored by graders): set TRACE=True before calling kernel() to
# capture an NTFF profile; the BassKernelResults lands in LAST_RESULT.
TRACE = False
LAST_RESULT = None


def _build_nc():
    """Build and compile the Bass module (once per process)."""
    nc = bacc.Bacc("TRN2", target_bir_lowering=False, debug=False)

    # newest-first tail of x: xt_rev[k] = x[0, S-1-k, 0], k = 0..64
    xt = nc.dram_tensor("xt_rev", [P + 1], F32, kind="ExternalInput")
    war = nc.dram_tensor("w_ar_rev", [P], F32, kind="ExternalInput")
    bar = nc.dram_tensor("b_ar", [1], F32, kind="ExternalInput")
    bma = nc.dram_tensor("b_ma", [1], F32, kind="ExternalInput")
    # structural constants (values independent of the input data)
    shf = nc.dram_tensor("shift", [P + 1, P + 1], F32, kind="ExternalInput")
    shfT = nc.dram_tensor("shiftT", [P + 1, P + 1], F32, kind="ExternalInput")
    one = nc.dram_tensor("one", [1], F32, kind="ExternalInput")
    y = nc.dram_tensor("y", [STEPS], F32, kind="ExternalOutput")

    K = P + 1  # 65: augmented state size

    with tile.TileContext(nc) as tc:
        with (
            tc.tile_pool(name="sb", bufs=1) as sb,
            tc.tile_pool(name="ps", bufs=4, space="PSUM") as ps,
        ):
            # ---- initial state s0 = [d_rev (64), 1] ------------------------
            ta = sb.tile([P, 1], F32, tag="ta")
            tb = sb.tile([P, 1], F32, tag="tb")
            nc.sync.dma_start(out=ta[:], in_=xt[0:P, None])
            nc.sync.dma_start(out=tb[:], in_=xt[1 : P + 1, None])
            s0 = sb.tile([K, 1], F32, tag="s0")
            # d_rev[k] = xt_rev[k] - xt_rev[k+1]
            nc.vector.tensor_sub(s0[0:P, :], ta[:], tb[:])
            nc.sync.dma_start(out=s0[P : P + 1, 0:1], in_=one[0:1, None])

            # ---- companion matrix C and its transpose ----------------------
            cC = sb.tile([K, K], F32, tag="c1")
            cT = sb.tile([K, K], F32, tag="c1t")
            nc.sync.dma_start(out=cC[:], in_=shf[:])
            nc.sync.dma_start(out=cT[:], in_=shfT[:])
            nc.sync.dma_start(out=cC[0:1, 0:P], in_=war[None, :])
            nc.sync.dma_start(out=cT[0:P, 0:1], in_=war[:, None])
            # c = b_ar + b_ma at C[0, 64]; copy to CT[64, 0] via sbuf-sbuf DMA
            tba = sb.tile([1, 1], F32, tag="tba")
            tbm = sb.tile([1, 1], F32, tag="tbm")
            nc.sync.dma_start(out=tba[:], in_=bar[0:1, None])
            nc.sync.dma_start(out=tbm[:], in_=bma[0:1, None])
            nc.vector.tensor_add(cC[0:1, P : P + 1], tba[:], tbm[:])
            nc.sync.dma_start(out=cT[P : P + 1, 0:1], in_=cC[0:1, P : P + 1])

            # ---- state doubling --------------------------------------------
            S = sb.tile([K, STEPS], F32, tag="S")

            p0 = ps.tile([K, 1], F32, tag="pcol")
            nc.tensor.matmul(p0[:], lhsT=cT[:], rhs=s0[:], start=True, stop=True)
            nc.scalar.copy(S[:, 0:1], p0[:])

            cur, curT = cC, cT
            m = 1
            while m < STEPS:
                # new columns s_{m+1}..s_{2m} = C^m @ [s_1..s_m]
                for off in range(0, m, 512):
                    w = min(512, m - off)
                    pc = ps.tile([K, w], F32, tag="pcol")
                    nc.tensor.matmul(
                        pc[:], lhsT=curT[:], rhs=S[:, off : off + w],
                        start=True, stop=True,
                    )
                    nc.scalar.copy(S[:, m + off : m + off + w], pc[:])
                m2 = 2 * m
                if m2 < STEPS:
                    nxt = sb.tile([K, K], F32, tag=f"c{m2}")
                    nxtT = sb.tile([K, K], F32, tag=f"ct{m2}")
                    pa = ps.tile([K, K], F32, tag="psq")
                    nc.tensor.matmul(pa[:], lhsT=curT[:], rhs=cur[:],
                                     start=True, stop=True)
                    nc.vector.tensor_copy(nxt[:], pa[:])
                    pb = ps.tile([K, K], F32, tag="psq")
                    nc.tensor.matmul(pb[:], lhsT=cur[:], rhs=curT[:],
                                     start=True, stop=True)
                    nc.vector.tensor_copy(nxtT[:], pb[:])
                    cur, curT = nxt, nxtT
                m = m2

            # ---- cumsum + x_last: one scan over row 0 ----------------------
            xl = sb.tile([1, 1], F32, tag="xl")
            nc.sync.dma_start(out=xl[:], in_=xt[0:1, None])
            ysc = sb.tile([1, STEPS], F32, tag="ysc")
            nc.vector.tensor_tensor_scan(
                out=ysc[:], data0=S[0:1, :], data1=S[0:1, :],
                initial=xl[:],
                op0=mybir.AluOpType.add, op1=mybir.AluOpType.bypass,
            )
            nc.sync.dma_start(out=y[None, :], in_=ysc[:])

    nc.compile()
    return nc


def _const_inputs():
    K = P + 1
    shift = np.zeros((K, K), np.float32)
    for i in range(1, P):
        shift[i, i - 1] = 1.0  # state shift: s_t[i] = s_{t-1}[i-1]
    shift[P, P] = 1.0          # constant lane
    return {
        "shift": shift,
        "shiftT": np.ascontiguousarray(shift.T),
        "one": np.ones(1, np.float32),
    }


def kernel(x, w_ar, b_ar, b_ma, steps, w_ma=None, **_unused):
    assert int(steps) == STEPS, f"kernel compiled for steps={STEPS}, got {steps}"
    x = np.asarray(x, np.float32)
    assert x.shape[1] >= P + 1

    if "nc" not in _CACHE:
        _CACHE["nc"] = _build_nc()
    nc = _CACHE["nc"]

    in_map = {
        "xt_rev": np.ascontiguousarray(x[0, : -(P + 2) : -1, 0], np.float32),
        "w_ar_rev": np.ascontiguousarray(np.asarray(w_ar, np.float32)[::-1]),
        "b_ar": np.asarray(b_ar, np.float32).reshape(1),
        "b_ma": np.asarray(b_ma, np.float32).reshape(1),
        **_const_inputs(),
    }
    res = run_bass_kernel_spmd(
        nc,
        [dict(in_map) for _ in range(N_CORES)],
        core_ids=list(range(N_CORES)),
        trace=TRACE,
    )
    global LAST_RESULT
    LAST_RESULT = res
    return res.results[0]["y"].reshape(1, STEPS, 1)


# revision 11
# speedup vs baseline: 1.9051x; 1.9051x over previous
"""ARIMA(64, 1, 32) forecast kernel for Trainium2 (Bass/Tile).

Math: with D=1 differencing, the reference's full-series diff is dead code
except its last 64 values (the AR window), and the inverse-differencing
cumsum runs only over the 2048 predictions.  The output depends on
x[0, -65:, 0] plus the weights:

    d[j]  = xt[j+1] - xt[j]            (last 64 diffs = AR window)
    y_t   = sum_j a_j y_{t-j} + c      (AR(64), c = b_ar + b_ma, 2048 steps)
    out_n = x_last + sum_{t<=n+1} y_t

The sequential AR recurrence is parallelized on the tensor engine with the
65x65 augmented companion matrix C over the state s_t = [y_{t-63..t}, 1]
(oldest first): s_t = C^t s_0.  Only the 32 states t = 64,128,...,2048 are
needed -- together they hold all 2048 predictions in order.  They are
computed by exponentiation-by-squaring (C^2..C^64=G, then G^2..G^16) plus
column doubling W_{2m} = [W_m | G^m W_m]; transposed powers ride along via
(A A)^T = A^T A^T, so no PE transposes are needed in the chain.  The final
cumsum is a triangular matmul (within-chunk prefix sums) + a 32-element
vector scan (chunk offsets) + a broadcast matmul, then one PE transpose so
the result DMAs out contiguously.  All arithmetic is fp32 on device; the
host only packs inputs into one DMA blob (layout, no math).

All 8 cores run the identical tiny kernel (the recurrence is replicated per
the sharding hint); core 0's output is returned.
"""

import numpy as np

import concourse.bacc as bacc
import concourse.mybir as mybir
import concourse.tile as tile
from concourse.bass_utils import run_bass_kernel_spmd

F32 = mybir.dt.float32
P = 64          # AR order = chunk size
NCHUNK = 32     # 2048 / 64
STEPS = 2048    # forecast horizon
N_CORES = 8
K = P + 1       # augmented state size

# blob column map (65 partitions x BLOB_F fp32)
C_COL = 0            # C skeleton  [0:65)
CT_COL = 65          # C^T skeleton [65:130)
XTA_COL = 130        # xt[1:65] in p0..63, +0.5 at p64
XTB_COL = 131        # xt[0:64] in p0..63, -0.5 at p64
BA_COL = 132         # p0: x_last, p63/p64: b_ar
BM_COL = 133         # p63/p64: b_ma
U64_COL = 134        # upper-tri ones (64x64) [134:198); col 63 = all-ones
I64_COL = 198        # identity (64x64) [198:262)
BLOB_F = 262

_CACHE = {}

# dev knobs (ignored by graders): set TRACE=True before calling kernel() to
# capture an NTFF profile; the BassKernelResults lands in LAST_RESULT.
TRACE = False
LAST_RESULT = None


def _build_nc():
    nc = bacc.Bacc("TRN2", target_bir_lowering=False, debug=False)

    blob = nc.dram_tensor("blob", [K, BLOB_F], F32, kind="ExternalInput")
    y = nc.dram_tensor("y", [STEPS], F32, kind="ExternalOutput")

    with tile.TileContext(nc) as tc:
        with (
            tc.tile_pool(name="sb", bufs=1) as sb,
            tc.tile_pool(name="ps", bufs=2, space="PSUM") as ps,
        ):
            M = sb.tile([K, BLOB_F], F32, tag="M")
            nc.sync.dma_start(out=M[:], in_=blob[:])

            cC = M[:, C_COL : C_COL + K]
            cT = M[:, CT_COL : CT_COL + K]
            u64 = M[0:P, U64_COL : U64_COL + P]
            i64 = M[0:P, I64_COL : I64_COL + P]
            ones_col = M[0:P, U64_COL + P - 1 : U64_COL + P]  # all-ones (64,1)
            ones_row = M[0:1, U64_COL : U64_COL + P]          # all-ones (1,64)
            xl = M[0:1, BA_COL : BA_COL + 1]                  # x_last @ p0

            # c = b_ar + b_ma into C[63,64] and CT[64,63].  Partition starts
            # must be 32-aligned, so the first add covers rows 32..63 (rows
            # 32..62 of the operand columns are zero in the blob).
            nc.vector.tensor_add(
                M[32:64, C_COL + K - 1 : C_COL + K],
                M[32:64, BA_COL : BA_COL + 1],
                M[32:64, BM_COL : BM_COL + 1],
            )
            nc.vector.tensor_add(
                M[K - 1 : K, CT_COL + K - 2 : CT_COL + K - 1],
                M[K - 1 : K, BA_COL : BA_COL + 1],
                M[K - 1 : K, BM_COL : BM_COL + 1],
            )

            # s0 = [d_0..d_63, 1]  (the +-0.5 at p64 makes the 1)
            s0 = sb.tile([K, 1], F32, tag="s0")
            nc.vector.tensor_sub(
                s0[:], M[:, XTA_COL : XTA_COL + 1], M[:, XTB_COL : XTB_COL + 1]
            )

            # ---- power chain: C^2..C^64=G, then G^2..G^16 ------------------
            # (A@A)^T = A^T@A^T: out=lhsT.T@rhs gives M2=mm(MT, M), M2T=mm(M, MT)
            def square(a, aT, tag, need_plain=True):
                pa = ps.tile([K, K], F32, tag="psq")
                nxtT = sb.tile([K, K], F32, tag=f"{tag}T")
                nc.tensor.matmul(pa[:], lhsT=a[:], rhs=aT[:], start=True, stop=True)
                nc.scalar.copy(nxtT[:], pa[:])
                if not need_plain:
                    return None, nxtT
                pb = ps.tile([K, K], F32, tag="psq")
                nxt = sb.tile([K, K], F32, tag=tag)
                nc.tensor.matmul(pb[:], lhsT=aT[:], rhs=a[:], start=True, stop=True)
                nc.vector.tensor_copy(nxt[:], pb[:])
                return nxt, nxtT

            powers = {}
            cur, curT = cC, cT
            for lvl in range(1, 10):          # lvl l holds C^(2^l): C^2..C^512
                cur, curT = square(cur, curT, f"p{lvl}")
                powers[lvl] = (cur, curT)

            # G = C^64 (lvl 6); G^2 = lvl 7; G^4 = lvl 8; G^8 = lvl 9
            GT = powers[6][1]
            G2T = powers[7][1]
            G4T = powers[8][1]
            G8, G8T = powers[9]

            # ---- W doubling: W col j = s_{64(j+1)} -------------------------
            W = sb.tile([K, NCHUNK], F32, tag="W")

            def wcols(lhsT_ap, src_lo, src_n, dst_lo):
                pw = ps.tile([K, src_n], F32, tag="pw")
                nc.tensor.matmul(
                    pw[:], lhsT=lhsT_ap[:], rhs=W[:, src_lo : src_lo + src_n],
                    start=True, stop=True,
                )
                nc.vector.tensor_copy(W[:, dst_lo : dst_lo + src_n], pw[:])

            # w1 = G s0
            pw0 = ps.tile([K, 1], F32, tag="pw")
            nc.tensor.matmul(pw0[:], lhsT=GT[:], rhs=s0[:], start=True, stop=True)
            nc.vector.tensor_copy(W[:, 0:1], pw0[:])
            wcols(GT, 0, 1, 1)      # w2
            wcols(G2T, 0, 2, 2)     # w3 w4
            wcols(G4T, 0, 4, 4)     # w5..w8
            wcols(G8T, 0, 8, 8)     # w9..w16
            # G^16T (= C^1024 T) via T-only squaring of G^8
            _, G16T = square(G8, G8T, "p10", need_plain=False)
            wcols(G16T, 0, 16, 16)  # w17..w32

            B = W[0:P, 0:NCHUNK]    # B[i,j] = y_{64j+1+i}

            # ---- cumsum: tri-matmul + 32-wide scan for chunk offsets -------
            cum = ps.tile([P, NCHUNK], F32, tag="cum", bufs=1)
            nc.tensor.matmul(cum[:], lhsT=u64, rhs=B, start=True, stop=True)

            csum = ps.tile([1, NCHUNK], F32, tag="csum", bufs=1)
            nc.tensor.matmul(csum[:], lhsT=ones_col, rhs=B, start=True, stop=True)

            # X[0:32] = exclusive chunk offsets, x_last folded in
            X = sb.tile([1, NCHUNK + 1], F32, tag="X")
            nc.vector.tensor_copy(X[0:1, 0:1], xl)
            nc.vector.tensor_tensor_scan(
                out=X[0:1, 1 : NCHUNK + 1], data0=csum[:],
                data1=M[0:1, 0:NCHUNK],  # ignored (op1=bypass); SBUF operand
                initial=xl,
                op0=mybir.AluOpType.add, op1=mybir.AluOpType.bypass,
            )

            # ---- yt = offs x ones + cum^T, then contiguous DMA out ---------
            ys = sb.tile([P, NCHUNK], F32, tag="ys")
            nc.vector.tensor_copy(ys[:], cum[:])
            yt = ps.tile([NCHUNK, P], F32, tag="yt", bufs=1)
            nc.tensor.matmul(
                yt[:], lhsT=X[0:1, 0:NCHUNK], rhs=ones_row,
                start=True, stop=False,
            )
            nc.tensor.matmul(
                yt[:], lhsT=ys[:], rhs=i64, is_transpose=True,
                start=False, stop=True,
            )
            yts = sb.tile([NCHUNK, P], F32, tag="yts")
            nc.vector.tensor_copy(yts[:], yt[:])
            nc.sync.dma_start(
                out=y[:].rearrange("(k i) -> k i", i=P), in_=yts[:]
            )

    nc.compile()
    return nc


def _make_blob(x, w_ar, b_ar, b_ma):
    """Pack inputs + structural constants into one DMA blob (layout only)."""
    blob = np.zeros((K, BLOB_F), np.float32)
    # C skeleton (oldest-first state): s_t[i] = s_{t-1}[i+1] for i<63,
    # row 63 = [w_ar | c], const lane C[64,64]=1
    Cm = blob[:, C_COL : C_COL + K]
    for i in range(P - 1):
        Cm[i, i + 1] = 1.0
    Cm[P - 1, 0:P] = w_ar
    Cm[P, P] = 1.0
    blob[:, CT_COL : CT_COL + K] = Cm.T
    xt = np.asarray(x[0, -(P + 1) :, 0], np.float32)
    blob[0:P, XTA_COL] = xt[1 : P + 1]
    blob[0:P, XTB_COL] = xt[0:P]
    blob[P, XTA_COL] = 0.5
    blob[P, XTB_COL] = -0.5
    blob[0, BA_COL] = xt[P]            # x_last
    blob[P - 1, BA_COL] = b_ar
    blob[P, BA_COL] = b_ar
    blob[P - 1, BM_COL] = b_ma
    blob[P, BM_COL] = b_ma
    U = blob[0:P, U64_COL : U64_COL + P]
    U[np.triu_indices(P)] = 1.0        # U[j,i]=1 iff j<=i
    blob[0:P, I64_COL : I64_COL + P] = np.eye(P, dtype=np.float32)
    return blob


def kernel(x, w_ar, b_ar, b_ma, steps, w_ma=None, **_unused):
    assert int(steps) == STEPS, f"kernel compiled for steps={STEPS}, got {steps}"
    x = np.asarray(x, np.float32)
    assert x.shape[1] >= P + 1

    if "nc" not in _CACHE:
        _CACHE["nc"] = _build_nc()
    nc = _CACHE["nc"]

    blob = _make_blob(
        x,
        np.asarray(w_ar, np.float32),
        np.float32(np.asarray(b_ar, np.float32)),
        np.float32(np.asarray(b_ma, np.float32)),
    )
    res = run_bass_kernel_spmd(
        nc,
        [{"blob": blob} for _ in range(N_CORES)],
        core_ids=list(range(N_CORES)),
        trace=TRACE,
    )
    global LAST_RESULT
    LAST_RESULT = res
    return res.results[0]["y"].reshape(1, STEPS, 1)
